# revision 35
# baseline (speedup 1.0000x reference)
"""Trainium2 Bass kernel for nn_DSVF (frequency-sampled SVF biquad, training path).

The reference applies H(z) = B(z)/A(z) (a biquad derived from 5 scalar params)
to each row of x via 8192-point FFT overlap-add on 4096-sample segments.  For
stable filters the segmented FFT application is numerically identical
(<< fp32 eps) to the plain causal IIR run per row.  For the graded inputs
(g=0, R=0, m_*=1) the poles sit at |z|^2 = 0.181, so the impulse response
decays by 0.181 per 2 samples: h[10] ~ 1.5e-4, i.e. the IIR is numerically a
9-tap causal FIR (truncation error ~2e-4 << the 2e-2 tolerance).

A short causal FIR maps onto the (otherwise idle) TensorEngine as one banded
Toeplitz matmul.  Each row is blocked into non-overlapping P=128-sample
blocks; a pure block transpose puts time-within-block on the partition axis:

    y[b*P + m] = sum_j h[j] x[b*P + m - j]  =  sum_pi W[pi, m] * X[pi, b]

with X[pi, b] = x[b*P + pi] and W[pi, m] = h[m - pi] (lower-banded [128, 128]
stationary, loaded once per tile).  The j > m cross-block corner terms (8
samples per block boundary) are added on the host as 20 cheap strided
numpy AXPYs with exact f64 taps.

I/O runs in float8_e4m3 (MODE="fp8"): the device computes only the tail taps
h[2..8] (pre-scaled so the dominant tap h[2] lands exactly on the e4m3 grid),
and the host adds the h[0]*x term in full f32 - so fp8 quantization only
touches the small correction signal (|d| ~ 0.15 sigma), keeping rel err at
~5.8e-3 against the 2e-2 gate while quartering HBM traffic vs f32.

Measured on 8 TRN2 cores: ~40 us (vs 138 us for the DVE tensor_tensor_scan
predecessor, kernel_scan_backup.py).  Steady state is co-bound by PE matmul
columns (32768 cols/core at ~376 ns/512, HAM clock capped ~1.4 GHz here),
the two PSUM-readers (DVE+ACT casts, ~0.65 us per 1024 cols), and DMA
(~8.4 MB/core over two HWDGE rings); ~7 us is fixed NEFF/Tile preamble.

Sharding: pure data parallel - 8 rows of x per core across 8 cores.
"""

import math
import sys

import numpy as np
import ml_dtypes

for _p in ("/opt/trn_rl_repo",):
    if _p not in sys.path:
        sys.path.insert(0, _p)

N_CORES = 8
B_FULL = 64
T_FULL = 524288
ROWS = B_FULL // N_CORES   # 8 rows per core

P = 128                    # block size (partition dim / contraction dim)
LAG = 8                    # FIR reach; taps h[0..LAG]
M = P                      # outputs per block (cross-block terms fixed on host)
NB = T_FULL // P           # 4096 blocks per row
COLS = ROWS * NB           # 32768 free columns per core

PSUM_CHUNK = 1024          # columns per PSUM tile (2 banks)
MM_CHUNK = 512             # columns per matmul (1 PSUM bank)

# "fp8": device computes the tail taps h[2..LAG] in float8_e4m3 I/O and the
#        host adds the dominant h[0]*x term in f32 (rel err ~8e-3).
# "bf16": device computes the full FIR in bfloat16 I/O (rel err ~2.5e-3).
MODE = "fp8"

_PROG_CACHE: dict = {}


def _build_program(cols: int, dt_in_name: str, dt_out_name: str):
    import concourse.bass as bass  # noqa: F401
    import concourse.bacc as bacc
    import concourse.tile as tile
    from concourse import mybir

    dt_in = getattr(mybir.dt, dt_in_name)
    dt_out = getattr(mybir.dt, dt_out_name)
    f32 = mybir.dt.float32

    nc = bacc.Bacc("TRN2")
    x = nc.declare_dram_parameter("x", [P, cols], dt_in, isOutput=False)
    w = nc.declare_dram_parameter("w", [P, M], dt_in, isOutput=False)
    y = nc.declare_dram_parameter("y", [M, cols], dt_out, isOutput=True)

    free_tile = 4096

    # tile schedule: small leading tiles (compute starts sooner) and a split
    # tail (the final store drains sooner); everything 1024-aligned
    sizes = [1024, 3072]
    while sum(sizes) < cols - free_tile:
        sizes.append(free_tile)
    rem = cols - sum(sizes)
    if rem >= 4096:
        sizes.extend([rem - 2048, 1024, 1024])
    elif rem > 0:
        sizes.extend([rem // 2, rem - rem // 2] if rem >= 2048 else [rem])
    tiles = []
    c0 = 0
    for fw in sizes:
        tiles.append((c0, fw))
        c0 += fw
    assert c0 == cols

    keep_ldw = set()
    with tile.TileContext(nc) as tc:
        with tc.tile_pool(name="wpool", bufs=1) as wpool, \
             tc.tile_pool(name="xin", bufs=5) as xpool, \
             tc.tile_pool(name="yout", bufs=4) as ypool, \
             tc.tile_pool(name="ps", bufs=4, space="PSUM") as pspool:
            wt = wpool.tile([P, M], dt_in)
            # weights via SWDGE so the first input tile owns the first SP-ring
            # issue slot
            nc.gpsimd.dma_start(out=wt[:], in_=w[:, :])

            copy_flip = 0
            for (c0, fw) in tiles:
                xt = xpool.tile([P, free_tile], dt_in)
                nc.sync.dma_start(out=xt[:, :fw], in_=x[:, c0:c0 + fw])
                yt = ypool.tile([M, free_tile], dt_out)
                # one stationary load per tile; the implicit per-matmul
                # LDWEIGHTS pairs are stripped below (bacc moves excess
                # matmul waits onto the most recent ldweights, so the
                # explicit one must stay tile-local)
                ld = nc.tensor.ldweights(wt[:])
                keep_ldw.add(ld.ins.name)
                h0 = 0
                while h0 < fw:
                    hw = min(PSUM_CHUNK, fw - h0)
                    ps = pspool.tile([M, PSUM_CHUNK], f32, tag="psg")
                    c = 0
                    while c < hw:
                        cw = min(MM_CHUNK, hw - c)
                        nc.tensor.matmul(
                            ps[:, c:c + cw], wt[:], xt[:, h0 + c:h0 + c + cw],
                            start=True, stop=True,
                        )
                        c += cw
                    # PSUM -> SBUF (cast to output dtype); both engines drain
                    # concurrently, split by engine speed (DVE 0.96 GHz with
                    # 120cyc overhead vs ACT 1.2 GHz with 172cyc) so both
                    # finish together and the PSUM slot frees fastest
                    dw = (7 * hw) // 16
                    nc.vector.tensor_copy(yt[:, h0:h0 + dw], ps[:, :dw])
                    nc.scalar.copy(yt[:, h0 + dw:h0 + hw], ps[:, dw:hw])
                    copy_flip += 1
                    h0 += hw
                # outputs issue from the otherwise-idle GpSimd (SWDGE),
                # inputs from the SP HWDGE ring: different queues for the two
                # directions, and the ACT engine keeps its cycles for PSUM
                # copies instead of DMA descriptor dispatch
                nc.gpsimd.dma_start(out=y[:, c0:c0 + fw], in_=yt[:, :fw])

    # The rust add_instruction pairs every InstMatmult with its own
    # InstLdweights reloading the identical stationary (~185 ns each on the
    # PE queue).  The weights never change, so drop every pairing that isn't
    # one of our explicit per-tile loads.  The paired loads carry no
    # waits/updates (verified: Tile hangs sync on the matmult itself), so
    # removal is sync-neutral.
    from concourse import mybir as _mb
    for fn in nc.m.functions:
        for bb in fn.blocks:
            insts = bb.instructions
            if any(isinstance(i, _mb.InstLdweights) for i in insts):
                kept = []
                for i in insts:
                    if isinstance(i, _mb.InstLdweights) and i.name not in keep_ldw:
                        si = i.sync_info
                        if si is None or (not si.on_wait and not si.on_update):
                            continue
                    kept.append(i)
                bb.instructions = kept
    nc.finalize()
    return nc


def _get_program(cols=COLS, dt_in="bfloat16", dt_out="bfloat16"):
    key = (cols, dt_in, dt_out)
    if key not in _PROG_CACHE:
        _PROG_CACHE[key] = _build_program(cols, dt_in, dt_out)
    return _PROG_CACHE[key]


def _svf_coeffs(g, R, m_hp, m_bp, m_lp):
    gg = math.tan(math.pi * (1.0 / (1.0 + math.exp(-g))) / 2.0)
    Rr = math.log1p(math.exp(R))
    g2 = gg * gg
    b = (g2 * m_lp + gg * m_bp + m_hp,
         2.0 * g2 * m_lp - 2.0 * m_hp,
         g2 * m_lp - gg * m_bp + m_hp)
    a = (g2 + 2.0 * Rr * gg + 1.0,
         2.0 * g2 - 2.0,
         g2 - 2.0 * Rr * gg + 1.0)
    return b, a


def _impulse_response(b, a, n):
    """First n taps of the biquad b/a impulse response (float64)."""
    b0, b1, b2 = (v / a[0] for v in b)
    a1, a2 = a[1] / a[0], a[2] / a[0]
    h = np.zeros(n, np.float64)
    x_hist = [0.0, 0.0]
    y_hist = [0.0, 0.0]
    for t in range(n):
        xt = 1.0 if t == 0 else 0.0
        yt = b0 * xt + b1 * x_hist[0] + b2 * x_hist[1] - a1 * y_hist[0] - a2 * y_hist[1]
        h[t] = yt
        x_hist = [xt, x_hist[0]]
        y_hist = [yt, y_hist[0]]
    return h


def _reference_fallback(x, b, a):
    """Exact numpy replication of the reference FFT overlap-add (any params)."""
    N = 4096
    NFFT = 8192
    B_, T = x.shape
    segs = x.astype(np.float64).reshape(B_, -1, N)
    X = np.fft.rfft(segs, n=NFFT, axis=-1)
    H = np.fft.rfft(np.asarray(b, np.float64), n=NFFT) / np.fft.rfft(
        np.asarray(a, np.float64), n=NFFT
    )
    yf = np.fft.irfft(X * H, n=NFFT, axis=-1)
    first = yf[:, :, :N]
    if segs.shape[1] == 1:
        return first.reshape(B_, -1).astype(np.float32)
    overlap = yf[:, :-1, N : 2 * N]
    overlap_ext = np.pad(overlap, ((0, 0), (1, 0), (0, 0)))
    return (first + overlap_ext).reshape(B_, -1).astype(np.float32)


def _make_weight(h):
    """Lower-banded Toeplitz lhsT [P, M]: W[m - j, m] = h[j] (within-block
    terms only; the j > m cross-block corner is added on the host)."""
    W = np.zeros((P, M), np.float64)
    for m in range(M):
        for j in range(min(LAG, m) + 1):
            W[m - j, m] = h[j]
    return W


def _im2col_core(xrows: np.ndarray, np_dt) -> np.ndarray:
    """[rows, T] f32 -> [128, rows*NB] device layout in np_dt.

    Column r*NB + b, partition pi holds x[r, b*P + pi] (a pure block
    transpose - no duplication).
    """
    rows = xrows.shape[0]
    out = np.empty((P, rows * NB), dtype=np_dt)
    for r in range(rows):
        out[:, r * NB:(r + 1) * NB] = xrows[r].reshape(NB, P).T.astype(np_dt)
    return out


def _uncol_core(ydev: np.ndarray) -> np.ndarray:
    """[P, rows*NB] device output -> [rows, T] float32."""
    rows = ydev.shape[1] // NB
    out = np.empty((rows, T_FULL), np.float32)
    for r in range(rows):
        slab = np.asarray(ydev[:, r * NB:(r + 1) * NB], dtype=np.float32)
        out[r] = slab.T.reshape(-1)
    return out


def _add_corner_terms(y: np.ndarray, x: np.ndarray, h) -> None:
    """Add the cross-block terms the device omits: for outputs t = b*P + m
    with m < j <= LAG, y[t] += h[j] * x[t - j] (exact, float64 taps)."""
    for j in range(1, LAG + 1):
        hj = float(h[j])
        if hj == 0.0:
            continue
        for m in range(j):
            ys = y[:, P + m::P]
            xs = x[:, P + m - j::P]
            ys += np.float32(hj) * xs[:, :ys.shape[1]]


def run_device(x, h, trace=False, mode=None, **spmd_kwargs):
    """Run the FIR program on all 8 cores; returns (y_full_f32, BassKernelResults)."""
    from concourse.bass_utils import run_bass_kernel_spmd

    mode = MODE if mode is None else mode
    if mode == "bf16":
        np_dt = ml_dtypes.bfloat16
        nc = _get_program(COLS, "bfloat16", "bfloat16")
        Wq = _make_weight(h).astype(np_dt)
        h_dev = None
        scale = 1.0
    else:
        # fp8: the device computes only the tail taps; h[0] stays on the host
        # in f32.  A global scale aligns the dominant tail tap h[2] exactly
        # onto the e4m3 grid so weight quantization error is negligible.
        np_dt = ml_dtypes.float8_e4m3
        nc = _get_program(COLS, "float8e4", "float8e4")
        h_dev = np.array(h, np.float64).copy()
        h_dev[0] = 0.0
        jmax = int(np.argmax(np.abs(h_dev)))
        q = float(np.asarray(h_dev[jmax], np.float32).astype(np_dt))
        scale = q / h_dev[jmax] if h_dev[jmax] != 0.0 else 1.0
        Wq = _make_weight(h_dev * scale).astype(np.float32).astype(np_dt)

    in_maps = []
    for c in range(N_CORES):
        xcore = _im2col_core(x[c * ROWS:(c + 1) * ROWS], np_dt)
        in_maps.append({"x": xcore, "w": Wq})
    res = run_bass_kernel_spmd(
        nc, in_maps, list(range(N_CORES)), trace=trace, **spmd_kwargs
    )
    out = np.concatenate(
        [_uncol_core(res.results[i]["y"]) for i in range(N_CORES)], axis=0
    )
    if mode != "bf16":
        out *= np.float32(1.0 / scale)
        out += np.float32(h[0]) * x
    _add_corner_terms(out, x, h)
    return out, res


def kernel(x, g, R, m_hp, m_bp, m_lp):
    x = np.ascontiguousarray(np.asarray(x, dtype=np.float32))
    gv, Rv, hpv, bpv, lpv = (
        float(np.asarray(v).reshape(-1)[0]) for v in (g, R, m_hp, m_bp, m_lp)
    )
    b, a = _svf_coeffs(gv, Rv, hpv, bpv, lpv)
    h64 = _impulse_response(b, a, 64)
    head = float(np.sqrt(np.sum(h64[:LAG + 1] ** 2)))
    tail = float(np.sqrt(np.sum(h64[LAG + 1:] ** 2)))
    fast_ok = (
        x.shape == (B_FULL, T_FULL)
        and head > 1e-8
        and tail < 1e-3 * head
    )
    if not fast_ok:
        return _reference_fallback(x, b, a)
    out, _ = run_device(x, h64[:LAG + 1])
    return out


# revision 36
# speedup vs baseline: 1.0306x; 1.0306x over previous
"""Trainium2 Bass kernel for nn_DSVF (frequency-sampled SVF biquad, training path).

The reference applies H(z) = B(z)/A(z) (a biquad derived from 5 scalar params)
to each row of x via 8192-point FFT overlap-add on 4096-sample segments.  For
stable filters the segmented FFT application is numerically identical
(<< fp32 eps) to the plain causal IIR run per row.  For the graded inputs
(g=0, R=0, m_*=1) the poles sit at |z|^2 = 0.181, so the impulse response
decays by 0.181 per 2 samples: h[10] ~ 1.5e-4, i.e. the IIR is numerically a
9-tap causal FIR (truncation error ~2e-4 << the 2e-2 tolerance).

A short causal FIR maps onto the (otherwise idle) TensorEngine as one banded
Toeplitz matmul.  Each row is blocked into non-overlapping P=128-sample
blocks; a pure block transpose puts time-within-block on the partition axis:

    y[b*P + m] = sum_j h[j] x[b*P + m - j]  =  sum_pi W[pi, m] * X[pi, b]

with X[pi, b] = x[b*P + pi] and W[pi, m] = h[m - pi] (lower-banded [128, 128]
stationary, loaded once per tile).  The j > m cross-block corner terms (8
samples per block boundary) are added on the host as 20 cheap strided
numpy AXPYs with exact f64 taps.

I/O runs in float8_e4m3 (MODE="fp8"): the device computes only the tail taps
h[2..8] (pre-scaled so the dominant tap h[2] lands exactly on the e4m3 grid),
and the host adds the h[0]*x term in full f32 - so fp8 quantization only
touches the small correction signal (|d| ~ 0.15 sigma), keeping rel err at
~5.8e-3 against the 2e-2 gate while quartering HBM traffic vs f32.

Measured on 8 TRN2 cores: ~40 us (vs 138 us for the DVE tensor_tensor_scan
predecessor, kernel_scan_backup.py).  Steady state is co-bound by PE matmul
columns (32768 cols/core at ~376 ns/512, HAM clock capped ~1.4 GHz here),
the two PSUM-readers (DVE+ACT casts, ~0.65 us per 1024 cols), and DMA
(~8.4 MB/core over two HWDGE rings); ~7 us is fixed NEFF/Tile preamble.

Sharding: pure data parallel - 8 rows of x per core across 8 cores.
"""

import math
import sys

import numpy as np
import ml_dtypes

for _p in ("/opt/trn_rl_repo",):
    if _p not in sys.path:
        sys.path.insert(0, _p)

N_CORES = 8
B_FULL = 64
T_FULL = 524288
ROWS = B_FULL // N_CORES   # 8 rows per core

P = 128                    # block size (partition dim / contraction dim)
LAG = 8                    # FIR reach; taps h[0..LAG]
M = P                      # outputs per block (cross-block terms fixed on host)
NB = T_FULL // P           # 4096 blocks per row
COLS = ROWS * NB           # 32768 free columns per core

PSUM_CHUNK = 1024          # columns per PSUM tile (2 banks)
MM_CHUNK = 512             # columns per matmul (1 PSUM bank)

# "fp8": device computes the tail taps h[2..LAG] in float8_e4m3 I/O and the
#        host adds the dominant h[0]*x term in f32 (rel err ~8e-3).
# "bf16": device computes the full FIR in bfloat16 I/O (rel err ~2.5e-3).
MODE = "fp8"

_PROG_CACHE: dict = {}


def _build_program(cols: int, dt_in_name: str, dt_out_name: str):
    import concourse.bass as bass  # noqa: F401
    import concourse.bacc as bacc
    import concourse.tile as tile
    from concourse import mybir

    dt_in = getattr(mybir.dt, dt_in_name)
    dt_out = getattr(mybir.dt, dt_out_name)
    f32 = mybir.dt.float32

    nc = bacc.Bacc("TRN2")
    x = nc.declare_dram_parameter("x", [P, cols], dt_in, isOutput=False)
    w = nc.declare_dram_parameter("w", [P, M], dt_in, isOutput=False)
    y = nc.declare_dram_parameter("y", [M, cols], dt_out, isOutput=True)

    free_tile = 4096

    # tile schedule: small leading tiles (compute starts sooner) and a split
    # tail (the final store drains sooner); everything 1024-aligned
    sizes = [1024, 3072]
    while sum(sizes) < cols - free_tile:
        sizes.append(free_tile)
    rem = cols - sum(sizes)
    if rem >= 4096:
        sizes.extend([rem - 2048, 1024, 1024])
    elif rem > 0:
        sizes.extend([rem // 2, rem - rem // 2] if rem >= 2048 else [rem])
    tiles = []
    c0 = 0
    for fw in sizes:
        tiles.append((c0, fw))
        c0 += fw
    assert c0 == cols

    keep_ldw = set()
    with tile.TileContext(nc) as tc:
        with tc.tile_pool(name="wpool", bufs=1) as wpool, \
             tc.tile_pool(name="xin", bufs=6) as xpool, \
             tc.tile_pool(name="yout", bufs=6) as ypool, \
             tc.tile_pool(name="ps", bufs=4, space="PSUM") as pspool:
            wt = wpool.tile([P, M], dt_in)
            # weights via SWDGE so the first input tile owns the first SP-ring
            # issue slot
            nc.gpsimd.dma_start(out=wt[:], in_=w[:, :])

            copy_flip = 0
            for (c0, fw) in tiles:
                xt = xpool.tile([P, free_tile], dt_in)
                nc.sync.dma_start(out=xt[:, :fw], in_=x[:, c0:c0 + fw])
                yt = ypool.tile([M, free_tile], dt_out)
                # one stationary load per tile; the implicit per-matmul
                # LDWEIGHTS pairs are stripped below (bacc moves excess
                # matmul waits onto the most recent ldweights, so the
                # explicit one must stay tile-local)
                ld = nc.tensor.ldweights(wt[:])
                keep_ldw.add(ld.ins.name)
                h0 = 0
                while h0 < fw:
                    hw = min(PSUM_CHUNK, fw - h0)
                    ps = pspool.tile([M, PSUM_CHUNK], f32, tag="psg")
                    c = 0
                    while c < hw:
                        cw = min(MM_CHUNK, hw - c)
                        nc.tensor.matmul(
                            ps[:, c:c + cw], wt[:], xt[:, h0 + c:h0 + c + cw],
                            start=True, stop=True,
                        )
                        c += cw
                    # PSUM -> SBUF (cast to output dtype); both engines drain
                    # concurrently, split by engine speed (DVE 0.96 GHz with
                    # 120cyc overhead vs ACT 1.2 GHz with 172cyc) so both
                    # finish together and the PSUM slot frees fastest
                    dw = (7 * hw) // 16
                    nc.vector.tensor_copy(yt[:, h0:h0 + dw], ps[:, :dw])
                    nc.scalar.copy(yt[:, h0 + dw:h0 + hw], ps[:, dw:hw])
                    copy_flip += 1
                    h0 += hw
                # outputs issue from the otherwise-idle GpSimd (SWDGE),
                # inputs from the SP HWDGE ring: different queues for the two
                # directions, and the ACT engine keeps its cycles for PSUM
                # copies instead of DMA descriptor dispatch
                nc.gpsimd.dma_start(out=y[:, c0:c0 + fw], in_=yt[:, :fw])

    # The rust add_instruction pairs every InstMatmult with its own
    # InstLdweights reloading the identical stationary (~185 ns each on the
    # PE queue).  The weights never change, so drop every pairing that isn't
    # one of our explicit per-tile loads.  The paired loads carry no
    # waits/updates (verified: Tile hangs sync on the matmult itself), so
    # removal is sync-neutral.
    from concourse import mybir as _mb
    for fn in nc.m.functions:
        for bb in fn.blocks:
            insts = bb.instructions
            if any(isinstance(i, _mb.InstLdweights) for i in insts):
                kept = []
                for i in insts:
                    if isinstance(i, _mb.InstLdweights) and i.name not in keep_ldw:
                        si = i.sync_info
                        if si is None or (not si.on_wait and not si.on_update):
                            continue
                    kept.append(i)
                bb.instructions = kept
    nc.finalize()
    return nc


def _get_program(cols=COLS, dt_in="bfloat16", dt_out="bfloat16"):
    key = (cols, dt_in, dt_out)
    if key not in _PROG_CACHE:
        _PROG_CACHE[key] = _build_program(cols, dt_in, dt_out)
    return _PROG_CACHE[key]


def _svf_coeffs(g, R, m_hp, m_bp, m_lp):
    gg = math.tan(math.pi * (1.0 / (1.0 + math.exp(-g))) / 2.0)
    Rr = math.log1p(math.exp(R))
    g2 = gg * gg
    b = (g2 * m_lp + gg * m_bp + m_hp,
         2.0 * g2 * m_lp - 2.0 * m_hp,
         g2 * m_lp - gg * m_bp + m_hp)
    a = (g2 + 2.0 * Rr * gg + 1.0,
         2.0 * g2 - 2.0,
         g2 - 2.0 * Rr * gg + 1.0)
    return b, a


def _impulse_response(b, a, n):
    """First n taps of the biquad b/a impulse response (float64)."""
    b0, b1, b2 = (v / a[0] for v in b)
    a1, a2 = a[1] / a[0], a[2] / a[0]
    h = np.zeros(n, np.float64)
    x_hist = [0.0, 0.0]
    y_hist = [0.0, 0.0]
    for t in range(n):
        xt = 1.0 if t == 0 else 0.0
        yt = b0 * xt + b1 * x_hist[0] + b2 * x_hist[1] - a1 * y_hist[0] - a2 * y_hist[1]
        h[t] = yt
        x_hist = [xt, x_hist[0]]
        y_hist = [yt, y_hist[0]]
    return h


def _reference_fallback(x, b, a):
    """Exact numpy replication of the reference FFT overlap-add (any params)."""
    N = 4096
    NFFT = 8192
    B_, T = x.shape
    segs = x.astype(np.float64).reshape(B_, -1, N)
    X = np.fft.rfft(segs, n=NFFT, axis=-1)
    H = np.fft.rfft(np.asarray(b, np.float64), n=NFFT) / np.fft.rfft(
        np.asarray(a, np.float64), n=NFFT
    )
    yf = np.fft.irfft(X * H, n=NFFT, axis=-1)
    first = yf[:, :, :N]
    if segs.shape[1] == 1:
        return first.reshape(B_, -1).astype(np.float32)
    overlap = yf[:, :-1, N : 2 * N]
    overlap_ext = np.pad(overlap, ((0, 0), (1, 0), (0, 0)))
    return (first + overlap_ext).reshape(B_, -1).astype(np.float32)


def _make_weight(h):
    """Lower-banded Toeplitz lhsT [P, M]: W[m - j, m] = h[j] (within-block
    terms only; the j > m cross-block corner is added on the host)."""
    W = np.zeros((P, M), np.float64)
    for m in range(M):
        for j in range(min(LAG, m) + 1):
            W[m - j, m] = h[j]
    return W


def _im2col_core(xrows: np.ndarray, np_dt) -> np.ndarray:
    """[rows, T] f32 -> [128, rows*NB] device layout in np_dt.

    Column r*NB + b, partition pi holds x[r, b*P + pi] (a pure block
    transpose - no duplication).
    """
    rows = xrows.shape[0]
    out = np.empty((P, rows * NB), dtype=np_dt)
    for r in range(rows):
        out[:, r * NB:(r + 1) * NB] = xrows[r].reshape(NB, P).T.astype(np_dt)
    return out


def _uncol_core(ydev: np.ndarray) -> np.ndarray:
    """[P, rows*NB] device output -> [rows, T] float32."""
    rows = ydev.shape[1] // NB
    out = np.empty((rows, T_FULL), np.float32)
    for r in range(rows):
        slab = np.asarray(ydev[:, r * NB:(r + 1) * NB], dtype=np.float32)
        out[r] = slab.T.reshape(-1)
    return out


def _add_corner_terms(y: np.ndarray, x: np.ndarray, h) -> None:
    """Add the cross-block terms the device omits: for outputs t = b*P + m
    with m < j <= LAG, y[t] += h[j] * x[t - j] (exact, float64 taps)."""
    for j in range(1, LAG + 1):
        hj = float(h[j])
        if hj == 0.0:
            continue
        for m in range(j):
            ys = y[:, P + m::P]
            xs = x[:, P + m - j::P]
            ys += np.float32(hj) * xs[:, :ys.shape[1]]


def run_device(x, h, trace=False, mode=None, **spmd_kwargs):
    """Run the FIR program on all 8 cores; returns (y_full_f32, BassKernelResults)."""
    from concourse.bass_utils import run_bass_kernel_spmd

    mode = MODE if mode is None else mode
    if mode == "bf16":
        np_dt = ml_dtypes.bfloat16
        nc = _get_program(COLS, "bfloat16", "bfloat16")
        Wq = _make_weight(h).astype(np_dt)
        h_dev = None
        scale = 1.0
    else:
        # fp8: the device computes only the tail taps; h[0] stays on the host
        # in f32.  A global scale aligns the dominant tail tap h[2] exactly
        # onto the e4m3 grid so weight quantization error is negligible.
        np_dt = ml_dtypes.float8_e4m3
        nc = _get_program(COLS, "float8e4", "float8e4")
        h_dev = np.array(h, np.float64).copy()
        h_dev[0] = 0.0
        jmax = int(np.argmax(np.abs(h_dev)))
        q = float(np.asarray(h_dev[jmax], np.float32).astype(np_dt))
        scale = q / h_dev[jmax] if h_dev[jmax] != 0.0 else 1.0
        Wq = _make_weight(h_dev * scale).astype(np.float32).astype(np_dt)

    in_maps = []
    for c in range(N_CORES):
        xcore = _im2col_core(x[c * ROWS:(c + 1) * ROWS], np_dt)
        in_maps.append({"x": xcore, "w": Wq})
    res = run_bass_kernel_spmd(
        nc, in_maps, list(range(N_CORES)), trace=trace, **spmd_kwargs
    )
    out = np.concatenate(
        [_uncol_core(res.results[i]["y"]) for i in range(N_CORES)], axis=0
    )
    if mode != "bf16":
        out *= np.float32(1.0 / scale)
        out += np.float32(h[0]) * x
    _add_corner_terms(out, x, h)
    return out, res


def kernel(x, g, R, m_hp, m_bp, m_lp):
    x = np.ascontiguousarray(np.asarray(x, dtype=np.float32))
    gv, Rv, hpv, bpv, lpv = (
        float(np.asarray(v).reshape(-1)[0]) for v in (g, R, m_hp, m_bp, m_lp)
    )
    b, a = _svf_coeffs(gv, Rv, hpv, bpv, lpv)
    h64 = _impulse_response(b, a, 64)
    head = float(np.sqrt(np.sum(h64[:LAG + 1] ** 2)))
    tail = float(np.sqrt(np.sum(h64[LAG + 1:] ** 2)))
    fast_ok = (
        x.shape == (B_FULL, T_FULL)
        and head > 1e-8
        and tail < 1e-3 * head
    )
    if not fast_ok:
        return _reference_fallback(x, b, a)
    out, _ = run_device(x, h64[:LAG + 1])
    return out
